# revision 4
# baseline (speedup 1.0000x reference)
"""Trainium2 Bass kernel for nn_AdaptiveMask (dense MLP over upper-triangle gather).

Computation (reference):
    x_flat = x[:, iu0, iu1]                      # [B, M] strict-upper-tri gather
    h = relu(x_flat @ w1 + b1)                   # [B, H]
    m = sigmoid(h @ w2 + b2)                     # [B, M]
    W = scatter_sym(m); out = W * x              # [B, C, C]
    returns (out, m)

Strategy (8 NeuronCores):
  - Tensor-parallel over the hidden dim H: core c holds w1[:, cols_c] and
    w2[rows_c, :]; every core computes the full batch through its hidden slice,
    partial y = h_c @ w2_c is ReduceScatter-summed over the batch axis so core c
    ends with y for its 16 batches. Weights are read from HBM exactly once.
  - All matmuls in bf16 (fp32 PSUM accumulation); epilogue (sigmoid, gating
    multiplies) in fp32.
  - Biases are folded into the GEMMs (ones-column in x_flat picks up b1; a
    bias hidden-unit on core 7 injects b2 after the reduce).
  - Host does only layout (triangle gather/scatter, pad, shard, bf16 cast);
    every FLOP of the reference runs on device.
"""

import numpy as np
import ml_dtypes

import concourse.bass as bass
import concourse.bacc as bacc
import concourse.tile as tile
from concourse import mybir
from concourse import bass_utils

# ---- problem constants (must match reference.py's setup_inputs) ----
B, NCH = 128, 200
M, H = 19900, 9950
NCORES = 8
BL = B // NCORES  # 16 batches per core

K1, KT1 = 19968, 156  # GEMM1 contraction (19900 data + 1 bias row + pad), k-tiles
HC, KT2 = 1280, 10    # per-core hidden slots, GEMM2 k-tiles
M2 = 19968            # padded output columns (39 * 512)
H_PER = [1244] * 7 + [1242]
H_START = [0, 1244, 2488, 3732, 4976, 6220, 7464, 8708]
BIAS_SLOT = 1242      # core 7 local hidden slot for the b2 bias unit

W1_CHUNK = 8          # k-tiles per w1 DMA (2.6 MB bf16)
G2W = 2048            # GEMM2 DMA group width (4 n-chunks of 512)
NGRP = 10             # 9 * 2048 + 1536 = 19968
RS_W = [4096, 4096, 4096, 4096, 3584]  # ReduceScatter chunk widths

CDT = mybir.dt.bfloat16
NP_CDT = ml_dtypes.bfloat16
F32 = mybir.dt.float32

_IU = np.triu_indices(NCH, k=1)


def build_nc():
    nc = bacc.Bacc("TRN2", target_bir_lowering=False, debug=False, num_devices=NCORES)

    xT = nc.dram_tensor("xT", [K1, B], CDT, kind="ExternalInput")
    w1 = nc.dram_tensor("w1", [K1, HC], CDT, kind="ExternalInput")
    w2 = nc.dram_tensor("w2", [HC, M2], CDT, kind="ExternalInput")
    xf = nc.dram_tensor("xf", [BL, M2], F32, kind="ExternalInput")
    xtf = nc.dram_tensor("xtf", [BL, M2], F32, kind="ExternalInput")
    om = nc.dram_tensor("om", [BL, M2], F32, kind="ExternalOutput")
    ou = nc.dram_tensor("ou", [BL, M2], F32, kind="ExternalOutput")
    ol = nc.dram_tensor("ol", [BL, M2], F32, kind="ExternalOutput")

    with tile.TileContext(nc) as tc:
        with (
            tc.tile_pool(name="const", bufs=1) as constp,
            tc.tile_pool(name="hbuf", bufs=1) as hp,
        ):
            ident_dram = nc.inline_tensor(np.eye(128, dtype=NP_CDT), name="ident")
            ident = constp.tile([128, 128], CDT)
            nc.sync.dma_start(ident[:], ident_dram[:])

            # x_flat^T resident in SBUF: partition = k-within-tile, free = (ktile, batch)
            xT_sb = constp.tile([128, K1], CDT)
            nc.sync.dma_start(
                xT_sb[:].rearrange("p (k b) -> p k b", k=KT1),
                xT[:].rearrange("(k p) b -> p k b", p=128),
            )

            h_sb = hp.tile([128, HC], CDT)   # [batch, hidden_local]
            hT_sb = hp.tile([128, HC], CDT)  # [hidden_local, batch] as 10 k-tiles

            # ---------------- GEMM1: h = relu(x_flat @ w1_c + b1_c) ----------------
            with (
                tc.tile_pool(name="w1p", bufs=2) as w1p,
                tc.tile_pool(name="ps1", bufs=1, space="PSUM") as ps1,
                tc.tile_pool(name="pst", bufs=2, space="PSUM") as pst,
            ):
                n_chunks1 = [(0, 512), (512, 1024), (1024, 1280)]
                ph = [
                    ps1.tile([128, n1 - n0], F32, tag=f"ph{i}", name=f"ph{i}")
                    for i, (n0, n1) in enumerate(n_chunks1)
                ]
                n_w1_chunks = (KT1 + W1_CHUNK - 1) // W1_CHUNK
                for c in range(n_w1_chunks):
                    kc = min(W1_CHUNK, KT1 - c * W1_CHUNK)
                    w1t = w1p.tile([128, kc * HC], CDT, tag="w1t", name=f"w1t{c}")
                    nc.sync.dma_start(
                        w1t[:].rearrange("p (k f) -> p k f", k=kc),
                        w1[c * W1_CHUNK * 128 : (c * W1_CHUNK + kc) * 128, :].rearrange(
                            "(k p) f -> p k f", p=128
                        ),
                    )
                    for j in range(kc):
                        k = c * W1_CHUNK + j
                        for i, (n0, n1) in enumerate(n_chunks1):
                            nc.tensor.matmul(
                                ph[i][:, :],
                                xT_sb[:, k * 128 : (k + 1) * 128],
                                w1t[:, j * HC + n0 : j * HC + n1],
                                start=(k == 0),
                                stop=(k == KT1 - 1),
                            )
                for i, (n0, n1) in enumerate(n_chunks1):
                    nc.scalar.activation(
                        h_sb[:, n0:n1], ph[i][:, :], mybir.ActivationFunctionType.Relu
                    )
                # transpose h -> hT via PE (needed as GEMM2's stationary operand)
                for j in range(KT2):
                    pt = pst.tile([128, 128], CDT, tag="pt", name=f"pt{j}")
                    nc.tensor.transpose(pt[:], h_sb[:, j * 128 : (j + 1) * 128], ident[:])
                    nc.vector.tensor_copy(hT_sb[:, j * 128 : (j + 1) * 128], pt[:])

            # ---------------- GEMM2 + ReduceScatter + epilogue ----------------
            with (
                tc.tile_pool(name="w2p", bufs=2) as w2p,
                tc.tile_pool(name="ps2", bufs=2, space="PSUM") as ps2,
                tc.tile_pool(name="yp", bufs=2) as yp,
                tc.tile_pool(name="ep", bufs=2) as ep,
                tc.tile_pool(name="dram2", bufs=1, space="DRAM") as dram2,
            ):
                y_in = [dram2.tile([128, w], F32, tag=f"yin{r}", name=f"yin{r}") for r, w in enumerate(RS_W)]
                y_out = [dram2.tile([BL, w], F32, tag=f"yout{r}", name=f"yout{r}") for r, w in enumerate(RS_W)]

                for g in range(NGRP):
                    wg = G2W if g < NGRP - 1 else M2 - (NGRP - 1) * G2W
                    col0 = g * G2W
                    w2t = w2p.tile([128, KT2 * wg], CDT, tag="w2t", name=f"w2t{g}")
                    nc.sync.dma_start(
                        w2t[:].rearrange("p (k f) -> p k f", k=KT2),
                        w2[:, col0 : col0 + wg].rearrange("(k p) f -> p k f", p=128),
                    )
                    pg = ps2.tile([128, wg], F32, tag="pg", name=f"pg{g}")
                    for kk in range(KT2):
                        for n in range(wg // 512):
                            nc.tensor.matmul(
                                pg[:, n * 512 : (n + 1) * 512],
                                hT_sb[:, kk * 128 : (kk + 1) * 128],
                                w2t[:, kk * wg + n * 512 : kk * wg + (n + 1) * 512],
                                start=(kk == 0),
                                stop=(kk == KT2 - 1),
                            )
                    y_sb = yp.tile([128, wg], F32, tag="ysb", name=f"ysb{g}")
                    nc.vector.tensor_copy(y_sb[:], pg[:])
                    r, half = g // 2, g % 2
                    nc.sync.dma_start(y_in[r][:, half * G2W : half * G2W + wg], y_sb[:])
                    if half == 1:
                        wr = RS_W[r]
                        nc.gpsimd.collective_compute(
                            "ReduceScatter",
                            mybir.AluOpType.add,
                            replica_groups=[list(range(NCORES))],
                            ins=[y_in[r][:].opt()],
                            outs=[y_out[r][:].opt()],
                        )
                        # epilogue on own 16 batches, laid out [128, wr/8] with
                        # partition p = s*16 + b (s = column sub-block). SBUF-side
                        # APs stay plain 2D; the DRAM side carries the 3D pattern
                        # (dma_start only requires equal total sizes).
                        wl = wr // 8
                        col0r = r * 4096
                        yo = ep.tile([128, wl], F32, tag="yo", name=f"yo{r}")
                        nc.sync.dma_start(
                            yo[:], y_out[r][:].rearrange("b (s f) -> s b f", s=8)
                        )
                        xfs = ep.tile([128, wl], F32, tag="xfs", name=f"xfs{r}")
                        nc.sync.dma_start(
                            xfs[:],
                            xf[:, col0r : col0r + wr].rearrange("b (s f) -> s b f", s=8),
                        )
                        xtfs = ep.tile([128, wl], F32, tag="xtfs", name=f"xtfs{r}")
                        nc.sync.dma_start(
                            xtfs[:],
                            xtf[:, col0r : col0r + wr].rearrange("b (s f) -> s b f", s=8),
                        )
                        ms = ep.tile([128, wl], F32, tag="ms", name=f"ms{r}")
                        nc.scalar.activation(
                            ms[:], yo[:], mybir.ActivationFunctionType.Sigmoid
                        )
                        us = ep.tile([128, wl], F32, tag="us", name=f"us{r}")
                        nc.vector.tensor_mul(us[:], ms[:], xfs[:])
                        ls = ep.tile([128, wl], F32, tag="ls", name=f"ls{r}")
                        nc.vector.tensor_mul(ls[:], ms[:], xtfs[:])
                        for t, dst in ((ms, om), (us, ou), (ls, ol)):
                            nc.sync.dma_start(
                                dst[:, col0r : col0r + wr].rearrange(
                                    "b (s f) -> s b f", s=8
                                ),
                                t[:],
                            )

    nc.compile()
    return nc


def prep_in_maps(x, w1, b1, w2, b2):
    x = np.asarray(x)
    w1 = np.asarray(w1, dtype=np.float32)
    b1 = np.asarray(b1, dtype=np.float32)
    w2 = np.asarray(w2, dtype=np.float32)
    b2 = np.asarray(b2, dtype=np.float32)
    iu0, iu1 = _IU
    xfl = np.ascontiguousarray(x[:, iu0, iu1]).astype(np.float32)   # [B, M]
    xtfl = np.ascontiguousarray(x[:, iu1, iu0]).astype(np.float32)  # [B, M]

    xT = np.zeros((K1, B), dtype=NP_CDT)
    xT[:M] = xfl.T.astype(NP_CDT)
    xT[M] = 1.0  # bias-ones row: picks up b1 (and core 7's b2 unit)

    xf_p = np.zeros((B, M2), np.float32)
    xf_p[:, :M] = xfl
    xtf_p = np.zeros((B, M2), np.float32)
    xtf_p[:, :M] = xtfl

    in_maps = []
    for c in range(NCORES):
        h0, hn = H_START[c], H_PER[c]
        w1c = np.zeros((K1, HC), dtype=NP_CDT)
        w1c[:M, :hn] = w1[:, h0 : h0 + hn].astype(NP_CDT)
        w1c[M, :hn] = b1[h0 : h0 + hn].astype(NP_CDT)
        w2c = np.zeros((HC, M2), dtype=NP_CDT)
        w2c[:hn, :M] = w2[h0 : h0 + hn, :].astype(NP_CDT)
        if c == NCORES - 1:
            w1c[M, BIAS_SLOT] = 1.0  # h[:, BIAS_SLOT] = relu(1*1) = 1 on core 7 only
            w2c[BIAS_SLOT, :M] = b2.astype(NP_CDT)
        in_maps.append(
            {
                "xT": xT,
                "w1": w1c,
                "w2": w2c,
                "xf": np.ascontiguousarray(xf_p[c * BL : (c + 1) * BL]),
                "xtf": np.ascontiguousarray(xtf_p[c * BL : (c + 1) * BL]),
            }
        )
    return in_maps


def assemble(results):
    m = np.concatenate([results[c]["om"][:, :M] for c in range(NCORES)], axis=0)
    u = np.concatenate([results[c]["ou"][:, :M] for c in range(NCORES)], axis=0)
    l = np.concatenate([results[c]["ol"][:, :M] for c in range(NCORES)], axis=0)
    iu0, iu1 = _IU
    out = np.zeros((B, NCH, NCH), np.float32)
    out[:, iu0, iu1] = u
    out[:, iu1, iu0] = l
    return out.astype(np.float32), m.astype(np.float32)


_NC_CACHE = None


def kernel(x, w1, b1, w2, b2, _trace=False):
    global _NC_CACHE
    in_maps = prep_in_maps(x, w1, b1, w2, b2)
    if _NC_CACHE is None:
        _NC_CACHE = build_nc()
    res = bass_utils.run_bass_kernel_spmd(
        _NC_CACHE, in_maps, core_ids=list(range(NCORES)), trace=_trace
    )
    out = assemble(res.results)
    if _trace:
        return out, res
    return out


# revision 5
# speedup vs baseline: 1.2015x; 1.2015x over previous
"""Trainium2 Bass kernel for nn_AdaptiveMask (dense MLP over upper-triangle gather).

Computation (reference):
    x_flat = x[:, iu0, iu1]                      # [B, M] strict-upper-tri gather
    h = relu(x_flat @ w1 + b1)                   # [B, H]
    m = sigmoid(h @ w2 + b2)                     # [B, M]
    W = scatter_sym(m); out = W * x              # [B, C, C]
    returns (out, m)

Strategy (8 NeuronCores):
  - Tensor-parallel over the hidden dim H: core c holds w1[:, cols_c] and
    w2[rows_c, :]; every core computes the full batch through its hidden slice,
    partial y = h_c @ w2_c is ReduceScatter-summed over the batch axis so core c
    ends with y for its 16 batches. Weights are read from HBM exactly once.
  - All matmuls in bf16 (fp32 PSUM accumulation); the ReduceScatter also runs
    in bf16; epilogue (sigmoid, gating multiplies) in fp32.
  - Weights/x are pre-permuted on host into per-partition-contiguous layouts so
    every weight DMA is a plain 2D transfer at line rate.
  - Biases are folded into the GEMMs (ones-column in x_flat picks up b1; a
    bias hidden-unit on core 7 injects b2 after the reduce).
  - Host does only layout (triangle gather/scatter, pad, shard, permute, cast);
    every FLOP of the reference runs on device.
"""

import numpy as np
import ml_dtypes

import concourse.bass as bass
import concourse.bacc as bacc
import concourse.tile as tile
from concourse import mybir
from concourse import bass_utils

# ---- problem constants (must match reference.py's setup_inputs) ----
B, NCH = 128, 200
M, H = 19900, 9950
NCORES = 8
BL = B // NCORES  # 16 batches per core

K1, KT1 = 19968, 156  # GEMM1 contraction (19900 data + 1 bias row + pad), k-tiles
HC, KT2 = 1280, 10    # per-core hidden slots, GEMM2 k-tiles
M2 = 19968            # padded output columns (39 * 512)
H_PER = [1244] * 7 + [1242]
H_START = [0, 1244, 2488, 3732, 4976, 6220, 7464, 8708]
BIAS_SLOT = 1242      # core 7 local hidden slot for the b2 bias unit

W1_CHUNK = 8          # k-tiles per w1 DMA (2.6 MB bf16)
G2W = 2048            # GEMM2 group width (4 n-chunks of 512)
GRP_W = [2048] * 9 + [1536]                 # GEMM2 DMA group widths
# ReduceScatter chunks: which GEMM2 groups feed each chunk
RS_GROUPS = [(0, 1), (2, 3), (4, 5), (6, 7), (8,), (9,)]
RS_W = [4096, 4096, 4096, 4096, 2048, 1536]
RS_COL0 = [0, 4096, 8192, 12288, 16384, 18432]

CDT = mybir.dt.bfloat16
NP_CDT = ml_dtypes.bfloat16
F32 = mybir.dt.float32

_IU = np.triu_indices(NCH, k=1)


def build_nc():
    nc = bacc.Bacc("TRN2", target_bir_lowering=False, debug=False, num_devices=NCORES)

    # host-permuted layouts: per-partition-contiguous (see prep_in_maps)
    xT = nc.dram_tensor("xT", [128, K1], CDT, kind="ExternalInput")
    w1 = nc.dram_tensor("w1", [128, KT1 * HC], CDT, kind="ExternalInput")
    w2 = nc.dram_tensor("w2", [128, KT2 * M2], CDT, kind="ExternalInput")
    xf = nc.dram_tensor("xf", [BL, M2], F32, kind="ExternalInput")
    xtf = nc.dram_tensor("xtf", [BL, M2], F32, kind="ExternalInput")
    om = nc.dram_tensor("om", [BL, M2], F32, kind="ExternalOutput")
    ou = nc.dram_tensor("ou", [BL, M2], F32, kind="ExternalOutput")
    ol = nc.dram_tensor("ol", [BL, M2], F32, kind="ExternalOutput")

    with tile.TileContext(nc) as tc:
        with (
            tc.tile_pool(name="const", bufs=1) as constp,
            tc.tile_pool(name="hbuf", bufs=1) as hp,
        ):
            ident_dram = nc.inline_tensor(np.eye(128, dtype=NP_CDT), name="ident")
            ident = constp.tile([128, 128], CDT)
            nc.scalar.dma_start(ident[:], ident_dram[:])

            # x_flat^T resident in SBUF: partition = k-within-tile, free = (ktile, batch)
            xT_sb = constp.tile([128, K1], CDT)
            nc.scalar.dma_start(xT_sb[:], xT[:])

            h_sb = hp.tile([128, HC], CDT)   # [batch, hidden_local]
            hT_sb = hp.tile([128, HC], CDT)  # [hidden_local, batch] as 10 k-tiles

            # ---------------- GEMM1: h = relu(x_flat @ w1_c + b1_c) ----------------
            with (
                tc.tile_pool(name="w1p", bufs=3) as w1p,
                tc.tile_pool(name="ps1", bufs=1, space="PSUM") as ps1,
                tc.tile_pool(name="pst", bufs=2, space="PSUM") as pst,
            ):
                n_chunks1 = [(0, 512), (512, 1024), (1024, 1280)]
                ph = [
                    ps1.tile([128, n1 - n0], F32, tag=f"ph{i}", name=f"ph{i}")
                    for i, (n0, n1) in enumerate(n_chunks1)
                ]
                n_w1_chunks = (KT1 + W1_CHUNK - 1) // W1_CHUNK
                for c in range(n_w1_chunks):
                    kc = min(W1_CHUNK, KT1 - c * W1_CHUNK)
                    c0 = c * W1_CHUNK * HC
                    w1t = w1p.tile([128, kc * HC], CDT, tag="w1t", name=f"w1t{c}")
                    nc.sync.dma_start(w1t[:], w1[:, c0 : c0 + kc * HC])
                    for j in range(kc):
                        k = c * W1_CHUNK + j
                        for i, (n0, n1) in enumerate(n_chunks1):
                            nc.tensor.matmul(
                                ph[i][:, :],
                                xT_sb[:, k * 128 : (k + 1) * 128],
                                w1t[:, j * HC + n0 : j * HC + n1],
                                start=(k == 0),
                                stop=(k == KT1 - 1),
                            )
                for i, (n0, n1) in enumerate(n_chunks1):
                    nc.scalar.activation(
                        h_sb[:, n0:n1], ph[i][:, :], mybir.ActivationFunctionType.Relu
                    )
                # transpose h -> hT via PE (needed as GEMM2's stationary operand)
                for j in range(KT2):
                    pt = pst.tile([128, 128], CDT, tag="pt", name=f"pt{j}")
                    nc.tensor.transpose(pt[:], h_sb[:, j * 128 : (j + 1) * 128], ident[:])
                    nc.vector.tensor_copy(hT_sb[:, j * 128 : (j + 1) * 128], pt[:])

            # ---------------- GEMM2 + ReduceScatter(bf16) + epilogue ----------------
            with (
                tc.tile_pool(name="w2p", bufs=2) as w2p,
                tc.tile_pool(name="ps2", bufs=2, space="PSUM") as ps2,
                tc.tile_pool(name="yp", bufs=2) as yp,
                tc.tile_pool(name="ep", bufs=2) as ep,
                tc.tile_pool(name="dram2", bufs=1, space="DRAM") as dram2,
            ):
                y_in = [
                    dram2.tile([128, w], CDT, tag=f"yin{r}", name=f"yin{r}")
                    for r, w in enumerate(RS_W)
                ]
                y_out = [
                    dram2.tile([BL, w], CDT, tag=f"yout{r}", name=f"yout{r}")
                    for r, w in enumerate(RS_W)
                ]
                grp_off = np.cumsum([0] + [KT2 * w for w in GRP_W]).tolist()

                # group -> (rs chunk, column offset inside the chunk)
                g2rs = {}
                for r, gs in enumerate(RS_GROUPS):
                    off = 0
                    for g in gs:
                        g2rs[g] = (r, off)
                        off += GRP_W[g]

                for g in range(len(GRP_W)):
                    wg = GRP_W[g]
                    w2t = w2p.tile([128, KT2 * wg], CDT, tag="w2t", name=f"w2t{g}")
                    nc.sync.dma_start(w2t[:], w2[:, grp_off[g] : grp_off[g] + KT2 * wg])
                    pg = ps2.tile([128, wg], F32, tag="pg", name=f"pg{g}")
                    for kk in range(KT2):
                        for n in range(wg // 512):
                            nc.tensor.matmul(
                                pg[:, n * 512 : (n + 1) * 512],
                                hT_sb[:, kk * 128 : (kk + 1) * 128],
                                w2t[:, kk * wg + n * 512 : kk * wg + (n + 1) * 512],
                                start=(kk == 0),
                                stop=(kk == KT2 - 1),
                            )
                    y_sb = yp.tile([128, wg], CDT, tag="ysb", name=f"ysb{g}")
                    nc.vector.tensor_copy(y_sb[:], pg[:])  # f32 PSUM -> bf16 SBUF
                    r, roff = g2rs[g]
                    nc.scalar.dma_start(y_in[r][:, roff : roff + wg], y_sb[:])
                    if roff + wg == RS_W[r]:
                        wr = RS_W[r]
                        nc.gpsimd.collective_compute(
                            "ReduceScatter",
                            mybir.AluOpType.add,
                            replica_groups=[list(range(NCORES))],
                            ins=[y_in[r][:].opt()],
                            outs=[y_out[r][:].opt()],
                        )
                        # epilogue on own 16 batches, laid out [128, wr/8] with
                        # partition p = s*16 + b (s = column sub-block). SBUF-side
                        # APs stay plain 2D; the DRAM side carries the 3D pattern
                        # (dma_start only requires equal total sizes).
                        wl = wr // 8
                        col0r = RS_COL0[r]
                        yo = ep.tile([128, wl], CDT, tag="yo", name=f"yo{r}")
                        nc.scalar.dma_start(
                            yo[:], y_out[r][:].rearrange("b (s f) -> s b f", s=8)
                        )
                        xfs = ep.tile([128, wl], F32, tag="xfs", name=f"xfs{r}")
                        nc.scalar.dma_start(
                            xfs[:],
                            xf[:, col0r : col0r + wr].rearrange("b (s f) -> s b f", s=8),
                        )
                        xtfs = ep.tile([128, wl], F32, tag="xtfs", name=f"xtfs{r}")
                        nc.scalar.dma_start(
                            xtfs[:],
                            xtf[:, col0r : col0r + wr].rearrange("b (s f) -> s b f", s=8),
                        )
                        ms = ep.tile([128, wl], F32, tag="ms", name=f"ms{r}")
                        nc.scalar.activation(
                            ms[:], yo[:], mybir.ActivationFunctionType.Sigmoid
                        )
                        us = ep.tile([128, wl], F32, tag="us", name=f"us{r}")
                        nc.vector.tensor_mul(us[:], ms[:], xfs[:])
                        ls = ep.tile([128, wl], F32, tag="ls", name=f"ls{r}")
                        nc.vector.tensor_mul(ls[:], ms[:], xtfs[:])
                        for t, dst in ((ms, om), (us, ou), (ls, ol)):
                            nc.scalar.dma_start(
                                dst[:, col0r : col0r + wr].rearrange(
                                    "b (s f) -> s b f", s=8
                                ),
                                t[:],
                            )

    nc.compile()
    return nc


def prep_in_maps(x, w1, b1, w2, b2):
    x = np.asarray(x)
    w1 = np.asarray(w1, dtype=np.float32)
    b1 = np.asarray(b1, dtype=np.float32)
    w2 = np.asarray(w2, dtype=np.float32)
    b2 = np.asarray(b2, dtype=np.float32)
    iu0, iu1 = _IU
    xfl = np.ascontiguousarray(x[:, iu0, iu1]).astype(np.float32)   # [B, M]
    xtfl = np.ascontiguousarray(x[:, iu1, iu0]).astype(np.float32)  # [B, M]

    # xT permuted: xT[p, k*128 + b] = x_aug^T[k*128 + p, b]
    xTa = np.zeros((K1, B), dtype=NP_CDT)
    xTa[:M] = xfl.T.astype(NP_CDT)
    xTa[M] = 1.0  # bias-ones row: picks up b1 (and core 7's b2 unit)
    xTp = np.ascontiguousarray(
        xTa.reshape(KT1, 128, B).transpose(1, 0, 2).reshape(128, K1)
    )

    xf_p = np.zeros((B, M2), np.float32)
    xf_p[:, :M] = xfl
    xtf_p = np.zeros((B, M2), np.float32)
    xtf_p[:, :M] = xtfl

    in_maps = []
    for c in range(NCORES):
        h0, hn = H_START[c], H_PER[c]
        w1c = np.zeros((K1, HC), dtype=NP_CDT)
        w1c[:M, :hn] = w1[:, h0 : h0 + hn].astype(NP_CDT)
        w1c[M, :hn] = b1[h0 : h0 + hn].astype(NP_CDT)
        w2c = np.zeros((HC, M2), dtype=NP_CDT)
        w2c[:hn, :M] = w2[h0 : h0 + hn, :].astype(NP_CDT)
        if c == NCORES - 1:
            w1c[M, BIAS_SLOT] = 1.0  # h[:, BIAS_SLOT] = relu(1*1) = 1 on core 7 only
            w2c[BIAS_SLOT, :M] = b2.astype(NP_CDT)
        # permute: w1p[p, k*HC + f] = w1c[k*128 + p, f]
        w1p = np.ascontiguousarray(
            w1c.reshape(KT1, 128, HC).transpose(1, 0, 2).reshape(128, KT1 * HC)
        )
        # permute per GEMM2 group: [128, sum_g(10 * wg)]
        w2blocks = []
        col = 0
        for wg in GRP_W:
            blk = w2c[:, col : col + wg].reshape(KT2, 128, wg).transpose(1, 0, 2)
            w2blocks.append(blk.reshape(128, KT2 * wg))
            col += wg
        w2p = np.ascontiguousarray(np.concatenate(w2blocks, axis=1))
        in_maps.append(
            {
                "xT": xTp,
                "w1": w1p,
                "w2": w2p,
                "xf": np.ascontiguousarray(xf_p[c * BL : (c + 1) * BL]),
                "xtf": np.ascontiguousarray(xtf_p[c * BL : (c + 1) * BL]),
            }
        )
    return in_maps


def assemble(results):
    m = np.concatenate([results[c]["om"][:, :M] for c in range(NCORES)], axis=0)
    u = np.concatenate([results[c]["ou"][:, :M] for c in range(NCORES)], axis=0)
    l = np.concatenate([results[c]["ol"][:, :M] for c in range(NCORES)], axis=0)
    iu0, iu1 = _IU
    out = np.zeros((B, NCH, NCH), np.float32)
    out[:, iu0, iu1] = u
    out[:, iu1, iu0] = l
    return out.astype(np.float32), m.astype(np.float32)


_NC_CACHE = None


def kernel(x, w1, b1, w2, b2, _trace=False):
    global _NC_CACHE
    in_maps = prep_in_maps(x, w1, b1, w2, b2)
    if _NC_CACHE is None:
        _NC_CACHE = build_nc()
    res = bass_utils.run_bass_kernel_spmd(
        _NC_CACHE, in_maps, core_ids=list(range(NCORES)), trace=_trace
    )
    out = assemble(res.results)
    if _trace:
        return out, res
    return out
